# revision 19
# baseline (speedup 1.0000x reference)
"""Binarized 3-layer MLP on 8 TRN2 NeuronCores (data-parallel over batch).

Computation (matching the reference):
    h1  = x @ sign(W1).T          x: [65536, 784] fp32, W1: [400, 784]
    h2  = sign(h1) @ sign(W2).T   W2: [200, 400]
    out = sign(h2) @ sign(W3).T   W3: [10, 200]

Strategy:
  - Batch sharded 8192 rows/core; weights replicated.
  - All activations kept feature-major (features on SBUF partitions), so the
    contraction dim of every layer is already on partitions: no transposes.
  - Layer 1 precision: x is split into two fp16 components (hi = fp16(x),
    lo = fp16(x - hi)) whose sum reproduces x to ~2^-23 relative (the PE
    handles fp16 subnormals exactly; HW-measured max err 4e-6 vs fp64 at
    K=128, same as fp32 matmul). Since sign(W1) is exactly +-1 in fp16, the
    two accumulated fp16 matmuls give fp32-quality h1 at full PE speed. The
    two components are concatenated along K (784*2 -> padded 1664 = 13
    k-tiles).
  - Layers 2/3: sign() outputs are exactly representable in bf16 and PSUM
    accumulates in fp32, so plain bf16 matmuls are exact.
  - The 400-row layer-1 output tiles as 128+128+128+16. The 16-row remainder
    ("m4") would waste a full-width matmul per k-tile, so chunks are processed
    in groups of 4 and the four 16-row matmuls are packed into one PSUM bank
    at partition strips 0/32/64/96 via tile_position col-tiling; the hardware
    runs matmuls in distinct 32-column groups concurrently. Accumulation uses
    memset-to-zero + start=False (accumulate-onto-zero == overwrite for any
    stale has_written state), which keeps interleaved strip accumulation
    correct. Layer 3 (M=10) outputs are packed the same way.
"""

import numpy as np
import ml_dtypes

import concourse.bacc as bacc
import concourse.mybir as mybir
import concourse.tile as tile
from concourse.bass_utils import run_bass_kernel_spmd

BF16 = np.dtype(ml_dtypes.bfloat16)

NCORES = 8
B = 65536
BL = B // NCORES          # 8192 rows per core
D0, H1, H2, DO = 784, 400, 200, 10
CH = 512                  # batch columns per chunk (PSUM bank = 512 fp32)
NCH = BL // CH            # 16 chunks per core
GRP = 4                   # chunks per packing group
KT1 = 13                  # ceil(784*2 / 128) k-tiles for layer 1
K1P = KT1 * 128           # 1664 padded K for layer 1

_cache = {}


def _build():
    if "nc" in _cache:
        return _cache["nc"]

    f32 = mybir.dt.float32
    bf16 = mybir.dt.bfloat16
    f16 = mybir.dt.float16
    Sign = mybir.ActivationFunctionType.Sign

    nc = bacc.Bacc("TRN2", debug=False, num_devices=NCORES)

    d_x = nc.dram_tensor("xh", [NCH, 128, KT1, CH], f16, kind="ExternalInput").ap()
    # w1 split so the m1 slab (first matmuls) lands before the rest
    d_w1a = nc.dram_tensor("w1a", [128, KT1, 128], f16, kind="ExternalInput").ap()
    d_w1b = nc.dram_tensor("w1b", [128, KT1, H1 - 128], f16, kind="ExternalInput").ap()
    # w2 k-blocks 0..2 are features 0:384; block 3 holds features 384:400
    # replicated at partition strips 0/32/64/96 (matches packed a1_3 layout).
    d_w2 = nc.dram_tensor("w2", [128, 4, H2], bf16, kind="ExternalInput").ap()
    d_w3 = nc.dram_tensor("w3", [128, 2, DO], bf16, kind="ExternalInput").ap()
    d_out = nc.dram_tensor("out", [NCH, DO, CH], f32, kind="ExternalOutput").ap()

    m1sz = [128, 128, 128]     # full-width layer-1 m-tiles (m4 packed separately)
    k2sz = [128, 128, 128]     # layer-2 full k-tiles (k4=16 handled via strips)
    m2sz = [128, 72]
    k3sz = [128, 72]

    with tile.TileContext(nc) as tc:
        with (
            tc.tile_pool(name="wp", bufs=1) as wp,
            tc.tile_pool(name="xp", bufs=5) as xp,
            tc.tile_pool(name="ap_", bufs=2) as apool,
            tc.tile_pool(name="a2p", bufs=1) as a2pool,
            tc.tile_pool(name="op", bufs=2) as op,
            tc.tile_pool(name="ps1p", bufs=1, space="PSUM") as ps1p,
            tc.tile_pool(name="ps2p", bufs=1, space="PSUM") as ps2p,
            tc.tile_pool(name="ps3p", bufs=1, space="PSUM") as ps3p,
            tc.tile_pool(name="ps4p", bufs=2, space="PSUM") as ps4p,
        ):
            w1a = wp.tile([128, KT1, 128], f16, name="w1a")
            w1b = wp.tile([128, KT1, H1 - 128], f16, name="w1b")
            w2sb = wp.tile([128, 4, H2], bf16, name="w2sb")
            w3sb = wp.tile([128, 2, DO], bf16, name="w3sb")
            nc.sync.dma_start(out=w1a[:], in_=d_w1a)

            def w1_slice(k, m_off, m_sz):
                if m_off == 0:
                    return w1a[:, k, 0:m_sz]
                return w1b[:, k, m_off - 128 : m_off - 128 + m_sz]

            def layer1_m123(xch):
                """Full-width layer-1 m-tiles; returns [a1_m0, a1_m1, a1_m2].

                The last k-tile holds only 32 real K-rows, replicated host-side
                at partition strips 0/32/64 so the three m-tiles' tail matmuls
                run concurrently in distinct PE row-groups."""
                a1 = []
                pss = []
                for m in range(3):
                    ps = ps1p.tile([128, CH], f32, name=f"ps1_{m}")
                    for k in range(KT1 - 1):
                        nc.tensor.matmul(
                            ps[:],
                            w1_slice(k, m * 128, 128),
                            xch[:, k, :],
                            start=(k == 0),
                            stop=False,
                        )
                    pss.append(ps)
                kl = KT1 - 1
                for m in range(3):
                    s = 32 * m
                    lhsT = (
                        w1a[s : s + 32, kl, 0:128]
                        if m == 0
                        else w1b[s : s + 32, kl, (m - 1) * 128 : m * 128]
                    )
                    nc.tensor.matmul(
                        pss[m][:],
                        lhsT,
                        xch[s : s + 32, kl, :],
                        start=False,
                        stop=True,
                        tile_position=(s, 0),
                    )
                for m in range(3):
                    at = apool.tile([128, CH], bf16, name=f"a1_{m}")
                    nc.scalar.activation(at[:], pss[m][:], Sign)
                    a1.append(at)
                return a1

            def layer2(jj, a1m, a13p):
                """Layer 2 for chunk jj of the group; a13p is the packed
                (4-strip) a1 remainder tile. Returns [a2_m0, a2_m1]."""
                a2 = []
                for m in range(2):
                    sz = m2sz[m]
                    ps = ps2p.tile([sz, CH], f32, name=f"ps2_{m}")
                    for k in range(3):
                        nc.tensor.matmul(
                            ps[:],
                            w2sb[:, k, m * 128 : m * 128 + sz],
                            a1m[k][:],
                            start=(k == 0),
                            stop=False,
                        )
                    s = 32 * jj
                    nc.tensor.matmul(
                        ps[:],
                        w2sb[s : s + 16, 3, m * 128 : m * 128 + sz],
                        a13p[s : s + 16, :],
                        start=False,
                        stop=True,
                        tile_position=(s, 0),
                    )
                    at = a2pool.tile([sz, CH], bf16, name=f"a2_{jj}_{m}")
                    nc.scalar.activation(at[:], ps[:], Sign)
                    a2.append(at)
                return a2

            for g in range(NCH // GRP):
                xchs = []
                for jj in range(GRP):
                    xch = xp.tile([128, KT1, CH], f16, name="xch")
                    nc.sync.dma_start(out=xch[:], in_=d_x[g * GRP + jj])
                    xchs.append(xch)
                    if g == 0 and jj == 0:
                        nc.sync.dma_start(out=w1b[:], in_=d_w1b)
                    if g == 0 and jj == 1:
                        nc.sync.dma_start(out=w2sb[:], in_=d_w2)
                        nc.sync.dma_start(out=w3sb[:], in_=d_w3)

                # packed m4 PSUM bank: strips [32jj : 32jj+16] per chunk
                ps4 = ps4p.tile([128, CH], f32, name="ps4")
                nc.vector.memset(ps4[:], 0.0)

                a1s = [None] * GRP
                a1s[0] = layer1_m123(xchs[0])
                a1s[1] = layer1_m123(xchs[1])

                # m4 packed: 4 col-tiled strips, interleaved for concurrency
                for k in range(KT1):
                    kr = 32 if k == KT1 - 1 else 128  # real rows in tail tile
                    for jj in range(GRP):
                        s = 32 * jj
                        nc.tensor.matmul(
                            ps4[s : s + 16, :],
                            w1_slice(k, 384, 16)[0:kr],
                            xchs[jj][0:kr, k, :],
                            start=False,
                            stop=(k == KT1 - 1),
                            tile_position=(0, s),
                        )
                a13p = apool.tile([128, CH], bf16, name="a13p")
                nc.scalar.activation(a13p[:], ps4[:], Sign)

                a2s = [None] * GRP
                a2s[0] = layer2(0, a1s[0], a13p)
                a2s[1] = layer2(1, a1s[1], a13p)
                a1s[2] = layer1_m123(xchs[2])
                a2s[2] = layer2(2, a1s[2], a13p)
                a1s[3] = layer1_m123(xchs[3])
                a2s[3] = layer2(3, a1s[3], a13p)

                # layer 3, packed into one PSUM bank at strips [32jj:32jj+10]
                ps3 = ps3p.tile([128, CH], f32, name="ps3")
                nc.vector.memset(ps3[:], 0.0)
                for k in range(2):
                    ks = k3sz[k]
                    for jj in range(GRP):
                        s = 32 * jj
                        nc.tensor.matmul(
                            ps3[s : s + DO, :],
                            w3sb[0:ks, k, :],
                            a2s[jj][k][0:ks, :],
                            start=False,
                            stop=(k == 1),
                            tile_position=(0, s),
                        )
                osb = op.tile([128, CH], f32, name="osb")
                nc.vector.tensor_copy(osb[:], ps3[:])
                for jj in range(GRP):
                    s = 32 * jj
                    nc.sync.dma_start(
                        out=d_out[g * GRP + jj], in_=osb[s : s + DO, :]
                    )

    nc.compile()
    _cache["nc"] = nc
    return nc


def _prep_weights(W1, W2, W3):
    # [K, M] layouts, K on partitions, padded so K-tiles are uniform 128.
    w1T = np.sign(W1).T.astype(np.float32)  # [784, 400]
    w1cat = np.concatenate(
        [w1T, w1T, np.zeros((K1P - 2 * D0, H1), np.float32)], axis=0
    )  # [1664, 400]
    # replicate the 32-row K-tail at partition strips 32/64 of the last
    # k-tile (for row-packed concurrent tail matmuls)
    w1cat[1568:1600] = w1cat[1536:1568]
    w1cat[1600:1632] = w1cat[1536:1568]
    w1h = np.ascontiguousarray(
        w1cat.reshape(KT1, 128, H1).transpose(1, 0, 2)
    ).astype(np.float16)  # [128, 13, 400]
    w1ha = np.ascontiguousarray(w1h[:, :, 0:128])
    w1hb = np.ascontiguousarray(w1h[:, :, 128:H1])

    w2T = np.sign(W2).T.astype(np.float32)  # [400, 200]
    w2h = np.zeros((128, 4, H2), np.float32)
    for k in range(3):
        w2h[:, k, :] = w2T[k * 128 : (k + 1) * 128]
    for jj in range(GRP):
        w2h[32 * jj : 32 * jj + 16, 3, :] = w2T[384:400]
    w2h = w2h.astype(BF16)

    w3T = np.sign(W3).T.astype(np.float32)  # [200, 10]
    w3h = np.zeros((128, 2, DO), np.float32)
    w3h[:, 0, :] = w3T[0:128]
    w3h[0:72, 1, :] = w3T[128:200]
    w3h = w3h.astype(BF16)
    return w1ha, w1hb, w2h, w3h


def _prep_x_core(xc):
    # xc: [8192, 784] fp32 -> [16, 128, 13, 512] fp16 (hi/lo along K)
    xt = np.ascontiguousarray(xc.T.astype(np.float32))  # [784, 8192]
    hi = xt.astype(np.float16)
    lo = (xt - hi.astype(np.float32)).astype(np.float16)
    x2 = np.concatenate(
        [hi, lo, np.zeros((K1P - 2 * D0, BL), np.float16)], axis=0
    )  # [1664, 8192]
    x2[1568:1600] = x2[1536:1568]
    x2[1600:1632] = x2[1536:1568]
    return np.ascontiguousarray(
        x2.reshape(KT1, 128, NCH, CH).transpose(2, 1, 0, 3)
    )  # [16, 128, 13, 512]


def kernel(x, W1, W2, W3, _trace=False, **_kw):
    nc = _build()
    w1ha, w1hb, w2h, w3h = _prep_weights(
        np.asarray(W1, np.float32), np.asarray(W2, np.float32), np.asarray(W3, np.float32)
    )
    x = np.asarray(x, np.float32).reshape(B, D0)

    in_maps = []
    for c in range(NCORES):
        in_maps.append(
            {
                "xh": _prep_x_core(x[c * BL : (c + 1) * BL]),
                "w1a": w1ha,
                "w1b": w1hb,
                "w2": w2h,
                "w3": w3h,
            }
        )

    res = run_bass_kernel_spmd(nc, in_maps, core_ids=list(range(NCORES)), trace=_trace)

    out = np.empty((B, DO), np.float32)
    for c in range(NCORES):
        oc = res.results[c]["out"]  # [16, 10, 512]
        out[c * BL : (c + 1) * BL] = oc.transpose(0, 2, 1).reshape(BL, DO)
    if _trace:
        _cache["last_results"] = res
    return out


# revision 30
# speedup vs baseline: 1.0259x; 1.0259x over previous
"""Binarized 3-layer MLP on 8 TRN2 NeuronCores (data-parallel over batch).

Computation (matching the reference):
    h1  = x @ sign(W1).T          x: [65536, 784] fp32, W1: [400, 784]
    h2  = sign(h1) @ sign(W2).T   W2: [200, 400]
    out = sign(h2) @ sign(W3).T   W3: [10, 200]

Strategy:
  - Batch sharded 8192 rows/core; weights replicated.
  - All activations kept feature-major (features on SBUF partitions), so the
    contraction dim of every layer is already on partitions: no transposes.
  - Layer 1 precision: x is split into two fp16 components (hi = fp16(x),
    lo = fp16(x - hi)) whose sum reproduces x to ~2^-23 relative (the PE
    handles fp16 subnormals exactly; HW-measured max err 4e-6 vs fp64 at
    K=128, same as fp32 matmul). Since sign(W1) is exactly +-1 in fp16, the
    two accumulated fp16 matmuls give fp32-quality h1 at full PE speed. The
    two components are concatenated along K (784*2 -> padded 1664 = 13
    k-tiles).
  - Layers 2/3: sign() outputs are exactly representable in bf16 and PSUM
    accumulates in fp32, so plain bf16 matmuls are exact.
  - The 400-row layer-1 output tiles as 128+128+128+16. The 16-row remainder
    ("m4") would waste a full-width matmul per k-tile, so chunks are processed
    in groups of 4 and the four 16-row matmuls are packed into one PSUM bank
    at partition strips 0/32/64/96 via tile_position col-tiling; the hardware
    runs matmuls in distinct 32-column groups concurrently. Accumulation uses
    memset-to-zero + start=False (accumulate-onto-zero == overwrite for any
    stale has_written state), which keeps interleaved strip accumulation
    correct. Layer 3 (M=10) outputs are packed the same way.
"""

import contextlib
import ctypes
import os
import sys
import types

import numpy as np
import ml_dtypes

import concourse.bacc as bacc
import concourse.mybir as mybir
import concourse.tile as tile
from concourse.bass_utils import run_bass_kernel_spmd


def _ensure_axon_hooks():
    """concourse's trace path imports antenv.axon_hooks, which this image
    lacks; register a ctypes-backed stand-in so trace=True (or a stray
    BASS_TRACE=1 in the environment) cannot crash the run."""
    try:
        import antenv.axon_hooks  # noqa: F401
        return
    except ImportError:
        pass

    so_path = "/opt/axon/libaxon_pjrt.so"
    hook = None
    if os.path.exists(so_path):
        try:
            lib = ctypes.CDLL(so_path)
            if hasattr(lib, "axon_start_nrt_profile"):
                lib.axon_start_nrt_profile.argtypes = [
                    ctypes.POINTER(ctypes.c_int64),
                    ctypes.c_size_t,
                ]
                lib.axon_start_nrt_profile.restype = ctypes.c_int64
                lib.axon_stop_nrt_profile.argtypes = [ctypes.c_char_p]
                lib.axon_stop_nrt_profile.restype = ctypes.c_int64

                @contextlib.contextmanager
                def _hook(output_dir, device_ids):
                    import jax

                    jax.devices()
                    if device_ids:
                        ids = (ctypes.c_int64 * len(device_ids))(*device_ids)
                        rc = lib.axon_start_nrt_profile(ids, len(device_ids))
                    else:
                        rc = lib.axon_start_nrt_profile(None, 0)
                    if rc != 0:
                        raise RuntimeError(f"axon_start_nrt_profile rc={rc}")
                    try:
                        yield
                    finally:
                        lib.axon_stop_nrt_profile(str(output_dir).encode())

                hook = _hook
        except OSError:
            pass

    mod = types.ModuleType("antenv.axon_hooks")
    mod.get_axon_ntff_profile_hook = lambda: hook
    mod.set_axon_ntff_profile_hook = lambda h: None
    sys.modules["antenv.axon_hooks"] = mod

    import concourse.bass_utils as _bu

    _bu.upload_artifacts = lambda tmpdir: tmpdir

BF16 = np.dtype(ml_dtypes.bfloat16)

NCORES = 8
B = 65536
BL = B // NCORES          # 8192 rows per core
D0, H1, H2, DO = 784, 400, 200, 10
CH = 512                  # batch columns per chunk (PSUM bank = 512 fp32)
NCH = BL // CH            # 16 chunks per core
GRP = 4                   # chunks per packing group
KT1 = 13                  # ceil(784*2 / 128) k-tiles for layer 1
K1P = KT1 * 128           # 1664 padded K for layer 1

_cache = {}


def _build():
    if "nc" in _cache:
        return _cache["nc"]

    f32 = mybir.dt.float32
    bf16 = mybir.dt.bfloat16
    f16 = mybir.dt.float16
    Sign = mybir.ActivationFunctionType.Sign

    nc = bacc.Bacc("TRN2", debug=False, num_devices=NCORES)

    d_x = nc.dram_tensor("xh", [NCH, 128, KT1, CH], f16, kind="ExternalInput").ap()
    # w1 split so the m1 slab (first matmuls) lands before the rest
    d_w1a = nc.dram_tensor("w1a", [128, KT1, 128], f16, kind="ExternalInput").ap()
    d_w1b = nc.dram_tensor("w1b", [128, KT1, H1 - 128], f16, kind="ExternalInput").ap()
    # w2 k-blocks 0..2 are features 0:384; block 3 holds features 384:400
    # replicated at partition strips 0/32/64/96 (matches packed a1_3 layout).
    d_w2 = nc.dram_tensor("w2", [128, 4, H2], bf16, kind="ExternalInput").ap()
    d_w3 = nc.dram_tensor("w3", [128, 2, DO], bf16, kind="ExternalInput").ap()
    d_out = nc.dram_tensor("out", [NCH, DO, CH], f32, kind="ExternalOutput").ap()

    m1sz = [128, 128, 128]     # full-width layer-1 m-tiles (m4 packed separately)
    k2sz = [128, 128, 128]     # layer-2 full k-tiles (k4=16 handled via strips)
    m2sz = [128, 72]
    k3sz = [128, 72]

    with tile.TileContext(nc) as tc:
        with (
            tc.tile_pool(name="wp", bufs=1) as wp,
            tc.tile_pool(name="xp", bufs=6) as xp,
            tc.tile_pool(name="ap_", bufs=2) as apool,
            tc.tile_pool(name="a2p", bufs=2) as a2pool,
            tc.tile_pool(name="op", bufs=2) as op,
            tc.tile_pool(name="ps1p", bufs=1, space="PSUM") as ps1p,
            tc.tile_pool(name="ps2p", bufs=1, space="PSUM") as ps2p,
            tc.tile_pool(name="pspk", bufs=2, space="PSUM") as pspk,
        ):
            w1a = wp.tile([128, KT1, 128], f16, name="w1a")
            w1b = wp.tile([128, KT1, H1 - 128], f16, name="w1b")
            w2sb = wp.tile([128, 4, H2], bf16, name="w2sb")
            w3sb = wp.tile([128, 2, DO], bf16, name="w3sb")
            nc.sync.dma_start(out=w1a[:], in_=d_w1a)

            def w1_slice(k, m_off, m_sz):
                if m_off == 0:
                    return w1a[:, k, 0:m_sz]
                return w1b[:, k, m_off - 128 : m_off - 128 + m_sz]

            def layer1_m123(xch):
                """Full-width layer-1 m-tiles; returns [a1_m0, a1_m1, a1_m2].

                The last k-tile holds only 32 real K-rows, replicated host-side
                at partition strips 0/32/64 so the three m-tiles' tail matmuls
                run concurrently in distinct PE row-groups."""
                a1 = []
                pss = []
                for m in range(3):
                    ps = ps1p.tile(
                        [128, CH], f32, name=f"ps1_{m}", bufs=(2 if m == 0 else 1)
                    )
                    for k in range(KT1 - 1):
                        nc.tensor.matmul(
                            ps[:],
                            w1_slice(k, m * 128, 128),
                            xch[:, k, :],
                            start=(k == 0),
                            stop=False,
                        )
                    pss.append(ps)
                kl = KT1 - 1
                for m in range(3):
                    s = 32 * m
                    lhsT = (
                        w1a[s : s + 32, kl, 0:128]
                        if m == 0
                        else w1b[s : s + 32, kl, (m - 1) * 128 : m * 128]
                    )
                    nc.tensor.matmul(
                        pss[m][:],
                        lhsT,
                        xch[s : s + 32, kl, :],
                        start=False,
                        stop=True,
                        tile_position=(s, 0),
                    )
                for m in range(3):
                    at = apool.tile([128, CH], bf16, name=f"a1_{m}")
                    nc.scalar.activation(at[:], pss[m][:], Sign)
                    a1.append(at)
                return a1

            def layer2(jj, a1m, a13p):
                """Layer 2 for chunk jj of the group; a13p is the packed
                (4-strip) a1 remainder tile. Returns [a2_m0, a2_m1]."""
                a2 = [None, None]
                # alternate m order per chunk so consecutive chunks' same-m
                # groups are further apart (ps2 banks are single-buffered)
                for m in ((0, 1) if jj % 2 == 0 else (1, 0)):
                    sz = m2sz[m]
                    ps = ps2p.tile([sz, CH], f32, name=f"ps2_{m}")
                    for k in range(3):
                        nc.tensor.matmul(
                            ps[:],
                            w2sb[:, k, m * 128 : m * 128 + sz],
                            a1m[k][:],
                            start=(k == 0),
                            stop=False,
                        )
                    s = 32 * jj
                    nc.tensor.matmul(
                        ps[:],
                        w2sb[s : s + 16, 3, m * 128 : m * 128 + sz],
                        a13p[s : s + 16, :],
                        start=False,
                        stop=True,
                        tile_position=(s, 0),
                    )
                    at = a2pool.tile([sz, CH], bf16, name=f"a2_{jj}_{m}")
                    nc.scalar.activation(at[:], ps[:], Sign)
                    a2[m] = at
                return a2

            # HAM/P-state pre-warm: dummy matmuls on a scratch tile keep the
            # PE busy during the initial weight/x DMA wait so the first real
            # matmuls run at full clock (the activity window is ~3.4us).
            warm = wp.tile([128, 64], f16, name="warm")
            nc.vector.memset(warm[:], 1.0)
            wps = pspk.tile([64, 64], f32, name="wps", tag="pack")
            for _ in range(48):
                nc.tensor.matmul(wps[:], warm[:, 0:64], warm[:], start=True, stop=True)

            for g in range(NCH // GRP):
                xchs = []
                for jj in range(GRP):
                    xch = xp.tile([128, KT1, CH], f16, name="xch")
                    nc.sync.dma_start(out=xch[:], in_=d_x[g * GRP + jj])
                    xchs.append(xch)
                    if g == 0 and jj == 0:
                        nc.sync.dma_start(out=w1b[:], in_=d_w1b)
                    if g == 0 and jj == 1:
                        nc.sync.dma_start(out=w2sb[:], in_=d_w2)
                        nc.sync.dma_start(out=w3sb[:], in_=d_w3)

                # packed m4 PSUM bank: strips [32jj : 32jj+16] per chunk
                ps4 = pspk.tile([128, CH], f32, name="ps4", tag="pack")
                nc.vector.memset(ps4[:], 0.0)

                a1s = [None] * GRP
                a1s[0] = layer1_m123(xchs[0])
                a1s[1] = layer1_m123(xchs[1])

                # m4 packed: 4 col-tiled strips, interleaved for concurrency
                for k in range(KT1):
                    kr = 32 if k == KT1 - 1 else 128  # real rows in tail tile
                    for jj in range(GRP):
                        s = 32 * jj
                        nc.tensor.matmul(
                            ps4[s : s + 16, :],
                            w1_slice(k, 384, 16)[0:kr],
                            xchs[jj][0:kr, k, :],
                            start=False,
                            stop=(k == KT1 - 1),
                            tile_position=(0, s),
                        )
                a13p = apool.tile([128, CH], bf16, name="a13p")
                nc.scalar.activation(a13p[:], ps4[:], Sign)

                a2s = [None] * GRP
                a2s[0] = layer2(0, a1s[0], a13p)
                a2s[1] = layer2(1, a1s[1], a13p)
                a1s[2] = layer1_m123(xchs[2])
                a2s[2] = layer2(2, a1s[2], a13p)
                a1s[3] = layer1_m123(xchs[3])
                a2s[3] = layer2(3, a1s[3], a13p)

                # layer 3, packed into one PSUM bank at strips [32jj:32jj+10]
                ps3 = pspk.tile([128, CH], f32, name="ps3", tag="pack")
                nc.vector.memset(ps3[:], 0.0)
                for k in range(2):
                    ks = k3sz[k]
                    for jj in range(GRP):
                        s = 32 * jj
                        nc.tensor.matmul(
                            ps3[s : s + DO, :],
                            w3sb[0:ks, k, :],
                            a2s[jj][k][0:ks, :],
                            start=False,
                            stop=(k == 1),
                            tile_position=(0, s),
                        )
                osb = op.tile([128, CH], f32, name="osb")
                nc.vector.tensor_copy(osb[:], ps3[:])
                for jj in range(GRP):
                    s = 32 * jj
                    nc.sync.dma_start(
                        out=d_out[g * GRP + jj], in_=osb[s : s + DO, :]
                    )

    nc.compile()
    _cache["nc"] = nc
    return nc


def _prep_weights(W1, W2, W3):
    # [K, M] layouts, K on partitions, padded so K-tiles are uniform 128.
    w1T = np.sign(W1).T.astype(np.float32)  # [784, 400]
    w1cat = np.concatenate(
        [w1T, w1T, np.zeros((K1P - 2 * D0, H1), np.float32)], axis=0
    )  # [1664, 400]
    # replicate the 32-row K-tail at partition strips 32/64 of the last
    # k-tile (for row-packed concurrent tail matmuls)
    w1cat[1568:1600] = w1cat[1536:1568]
    w1cat[1600:1632] = w1cat[1536:1568]
    w1h = np.ascontiguousarray(
        w1cat.reshape(KT1, 128, H1).transpose(1, 0, 2)
    ).astype(np.float16)  # [128, 13, 400]
    w1ha = np.ascontiguousarray(w1h[:, :, 0:128])
    w1hb = np.ascontiguousarray(w1h[:, :, 128:H1])

    w2T = np.sign(W2).T.astype(np.float32)  # [400, 200]
    w2h = np.zeros((128, 4, H2), np.float32)
    for k in range(3):
        w2h[:, k, :] = w2T[k * 128 : (k + 1) * 128]
    for jj in range(GRP):
        w2h[32 * jj : 32 * jj + 16, 3, :] = w2T[384:400]
    w2h = w2h.astype(BF16)

    w3T = np.sign(W3).T.astype(np.float32)  # [200, 10]
    w3h = np.zeros((128, 2, DO), np.float32)
    w3h[:, 0, :] = w3T[0:128]
    w3h[0:72, 1, :] = w3T[128:200]
    w3h = w3h.astype(BF16)
    return w1ha, w1hb, w2h, w3h


def _prep_x_core(xc):
    # xc: [8192, 784] fp32 -> [16, 128, 13, 512] fp16 (hi/lo along K)
    xt = np.ascontiguousarray(xc.T.astype(np.float32))  # [784, 8192]
    hi = xt.astype(np.float16)
    lo = (xt - hi.astype(np.float32)).astype(np.float16)
    x2 = np.concatenate(
        [hi, lo, np.zeros((K1P - 2 * D0, BL), np.float16)], axis=0
    )  # [1664, 8192]
    x2[1568:1600] = x2[1536:1568]
    x2[1600:1632] = x2[1536:1568]
    return np.ascontiguousarray(
        x2.reshape(KT1, 128, NCH, CH).transpose(2, 1, 0, 3)
    )  # [16, 128, 13, 512]


def kernel(x, W1, W2, W3, _trace=False, **_kw):
    nc = _build()
    w1ha, w1hb, w2h, w3h = _prep_weights(
        np.asarray(W1, np.float32), np.asarray(W2, np.float32), np.asarray(W3, np.float32)
    )
    x = np.asarray(x, np.float32).reshape(B, D0)

    in_maps = []
    for c in range(NCORES):
        in_maps.append(
            {
                "xh": _prep_x_core(x[c * BL : (c + 1) * BL]),
                "w1a": w1ha,
                "w1b": w1hb,
                "w2": w2h,
                "w3": w3h,
            }
        )

    _ensure_axon_hooks()
    res = run_bass_kernel_spmd(nc, in_maps, core_ids=list(range(NCORES)), trace=_trace)

    out = np.empty((B, DO), np.float32)
    for c in range(NCORES):
        oc = res.results[c]["out"]  # [16, 10, 512]
        out[c * BL : (c + 1) * BL] = oc.transpose(0, 2, 1).reshape(BL, DO)
    if _trace:
        _cache["last_results"] = res
    return out
